# revision 21
# baseline (speedup 1.0000x reference)
"""DualAttention Trainium2 kernel.

Problem: x:[2,64,20,20,20]; three separable 1-D convs produce q0 (H-axis),
k0 (D-axis), v (W-axis), each [B,C,N] with N=8000; scores = k0^T q0 [B,N,N];
softmax over the key axis i (axis 1); out = v @ attn, reshaped back.

Sharding: 8 cores = 2 batches x 4 query-column slices of 2000. Each core
computes full k0/v (cheap convs) and its q0 slice, then a flash-style
scores->exp->accumulate loop. No collectives.

Per-core device algorithm (Tile framework):
  - convs as chunked matmuls: one K=128 matmul for the two shifted taps
    (host pre-stacks shifted x on 128 partitions), one K=64 matmul for the
    center tap. All convs in fp32; conv outputs k0/q0/v stored bf16.
  - scores: bf16 k0/q0 (fp32 PSUM accumulate); i-tile pairs run on disjoint
    PE row groups (k0/q0 duplicated across partition halves).
  - exp on ScalarE with bias -8 (uniform scale cancels in softmax), bf16 out.
  - second matmul: lhsT = [v^T | 1] bf16 [128,65]; accumulates numerator and
    softmax denominator (row 64) in one PSUM pass.
  - divide by denominator at the end (reciprocal + partition broadcast).
"""

import numpy as np

B, C, D, H, W = 2, 64, 20, 20, 20
N = D * H * W            # 8000
NCORES = 8
CPB = 4                  # cores per batch
MS = N // CPB            # 2000 query columns per core
ITILE = 128
NFULL = N // ITILE       # 62 full i-tiles
TAILW = N - NFULL * ITILE  # 64
NT = NFULL + 1           # 63
M_PASSES = [(0, 1024), (1024, 976)]
EXP_BIAS = -8.0

# Schraudolph-style exp on DVE: int16 bits = SCH_A*s + SCH_B viewed as bf16.
# bf16 bits = 128*log2(v) + 16256; log2(e^(s+EXP_BIAS)) = (s+EXP_BIAS)/ln2.
import math
SCH_A = 128.0 / math.log(2.0)
SCH_C = 7.4
SCH_B = 16256.0 - SCH_C + SCH_A * EXP_BIAS
DVE_TILES = (1, 3)  # t % 5 in this set -> exp on DVE (40%)

_CACHE = {}


def _build_bass(reps=1, bench_loop=0, passes=None):
    if passes is None:
        passes = M_PASSES
    import contextlib
    import concourse.tile as tile
    import concourse.mybir as mybir
    from concourse import bacc
    from concourse.masks import make_identity

    F32 = mybir.dt.float32
    F32R = mybir.dt.float32r
    I16 = mybir.dt.int16
    BF16 = mybir.dt.bfloat16
    EXP = mybir.ActivationFunctionType.Exp
    IDENT = mybir.ActivationFunctionType.Identity
    ADD = mybir.AluOpType.add
    MULT = mybir.AluOpType.mult

    nc = bacc.Bacc("TRN2", target_bir_lowering=False, debug=False,
                   num_devices=NCORES)

    xin_d = nc.dram_tensor("xin", [64, N], F32R, kind="ExternalInput")
    xD_d = nc.dram_tensor("xD", [128, N], F32R, kind="ExternalInput")
    xW_d = nc.dram_tensor("xW", [128, N], F32R, kind="ExternalInput")
    xH_d = nc.dram_tensor("xH", [128, MS], F32R, kind="ExternalInput")
    xq_d = nc.dram_tensor("xq", [64, MS], F32R, kind="ExternalInput")
    wp_d = nc.dram_tensor("wp", [128, 640], F32R, kind="ExternalInput")
    bp_d = nc.dram_tensor("bp", [128, 3], F32, kind="ExternalInput")
    out_d = nc.dram_tensor("out", [64, MS], F32, kind="ExternalOutput")

    PIECE = 2048  # input DMA piece size (512-aligned) for pipelining

    with tile.TileContext(nc) as tc:
        loop_cm = tc.For_i(0, bench_loop, 1) if bench_loop else contextlib.nullcontext()
        with loop_cm:
         for _rep in range(reps):
            with tc.tile_pool(name="persist", bufs=1) as per, \
                 tc.tile_pool(name="mps", bufs=2, space="PSUM") as mp, \
                 tc.tile_pool(name="ops", bufs=1, space="PSUM") as opp, \
                 tc.tile_pool(name="eps", bufs=10) as ep, \
                 tc.tile_pool(name="fin", bufs=1) as fin, \
                 tc.tile_pool(name="prep", bufs=1) as pr:

                w_sb = per.tile([128, 640], F32R, name="w_sb")
                nc.gpsimd.dma_start(w_sb[:, :], wp_d.ap())
                b_sb = per.tile([128, 3], F32, name="b_sb")
                nc.gpsimd.dma_start(b_sb[:, :], bp_d.ap())
                k0_sb = per.tile([128, N], BF16, name="k0_sb")
                q0_sb = per.tile([128, MS], BF16, name="q0_sb")
                v_sb = per.tile([64, N], BF16, name="v_sb")
                vT_sb = per.tile([128, NT * 65], BF16, name="vT_sb")
                out_sb = per.tile([64, MS], F32, name="out_sb")
                identb = per.tile([64, 64], BF16, name="identb")
                expb = per.tile([128, 1], F32, name="expb")

                make_identity(nc, identb[:, :])
                nc.vector.memset(expb[:, :], EXP_BIAS)
                nc.vector.memset(vT_sb[:, :], 1.0)
                vT_view = vT_sb[:, :].rearrange("p (t c) -> p t c", c=65)

                def conv_chunk(dst, wt, w0c, wpmc, bcol, s0_sb, sS_sb, c0, cw):
                    ps = mp.tile([128, 1024], F32, tag="s", name=f"c{w0c}_{c0}")
                    nc.tensor.matmul(ps[0:64, 0:cw],
                                     lhsT=wt[:, wpmc:wpmc + 64],
                                     rhs=sS_sb[:, c0:c0 + cw],
                                     start=True, stop=False)
                    nc.tensor.matmul(ps[0:64, 0:cw],
                                     lhsT=wt[0:64, w0c:w0c + 64],
                                     rhs=s0_sb[0:64, c0:c0 + cw],
                                     start=False, stop=True)
                    # bias-add + PSUM->SBUF move on ACT (Identity is in the
                    # exp_and_others table set: no table switch); keeps the
                    # DVE free for vT copies and hoisted exp during prep
                    nc.scalar.activation(dst[0:64, c0:c0 + cw],
                                         ps[0:64, 0:cw], IDENT,
                                         bias=b_sb[0:64, bcol:bcol + 1])

                # --- q conv first (small inputs, unblocks scores) ---
                xH_sb = pr.tile([128, MS], F32R, tag="xh", name="xH_sb")
                nc.sync.dma_start(xH_sb[:, :], xH_d.ap())
                xq_sb = pr.tile([64, MS], F32R, tag="xq", name="xq_sb")
                nc.gpsimd.dma_start(xq_sb[:, :], xq_d.ap())
                for c0 in range(0, MS, 512):
                    conv_chunk(q0_sb, w_sb, 256, 384, 1, xq_sb, xH_sb,
                               c0, min(512, MS - c0))
                nc.scalar.dma_start(q0_sb[64:128, :], q0_sb[0:64, :])

                # --- k conv, pipelined with pieced input DMA; per-chunk dup ---
                xin_sb = pr.tile([64, N], F32R, tag="xin", name="xin_sb")
                xD_sb = pr.tile([128, N], F32R, tag="xd", name="xD_sb")
                for p0 in range(0, N, PIECE):
                    pw = min(PIECE, N - p0)
                    nc.sync.dma_start(xD_sb[:, p0:p0 + pw], xD_d.ap()[:, p0:p0 + pw])
                    nc.gpsimd.dma_start(xin_sb[:, p0:p0 + pw],
                                        xin_d.ap()[:, p0:p0 + pw])
                xW_sb = pr.tile([128, N], F32R, tag="xw", name="xW_sb")
                for p0 in range(0, N, PIECE):
                    pw = min(PIECE, N - p0)
                    nc.scalar.dma_start(xW_sb[:, p0:p0 + pw], xW_d.ap()[:, p0:p0 + pw])
                for c0 in range(0, N, 512):
                    cw = min(512, N - c0)
                    conv_chunk(k0_sb, w_sb, 0, 128, 0, xin_sb, xD_sb, c0, cw)
                    nc.gpsimd.dma_start(k0_sb[64:128, c0:c0 + cw],
                                        k0_sb[0:64, c0:c0 + cw])

                def scores_exp(m0, mw, t, rows, name):
                    chunks = [(0, 512), (512, mw - 512)]
                    s = mp.tile([128, mw], F32, tag="s", name=f"s{name}")
                    for (c0, cw) in chunks:
                        nc.tensor.matmul(
                            s[0:rows, c0:c0 + cw],
                            lhsT=k0_sb[t % 2 * 64:t % 2 * 64 + 64,
                                       t * ITILE:t * ITILE + rows],
                            rhs=q0_sb[t % 2 * 64:t % 2 * 64 + 64,
                                      m0 + c0:m0 + c0 + cw],
                            start=True, stop=True)
                    e = ep.tile([128, mw], BF16, tag="e", name=f"e{name}")
                    if t % 5 in DVE_TILES:
                        nc.vector.tensor_scalar(
                            out=e[0:rows, :].bitcast(I16), in0=s[0:rows, :],
                            scalar1=SCH_A, scalar2=SCH_B, op0=MULT, op1=ADD)
                    else:
                        nc.scalar.activation(e[0:rows, :], s[0:rows, :], EXP,
                                             bias=expb[0:rows, :])
                    return e

                def out_mms(poa, pob, mw, t, rows, e, first_a, first_b,
                            last_a, last_b):
                    # K=128 contraction split into two K=64 row-group halves
                    # that run concurrently on the PE; accumulated in two
                    # separate PSUM banks, merged at normalize time.
                    for (c0, cw) in [(0, 512), (512, mw - 512)]:
                        nc.tensor.matmul(poa[:, c0:c0 + cw],
                                         lhsT=vT_view[0:64, t, :],
                                         rhs=e[0:64, c0:c0 + cw],
                                         start=first_a, stop=last_a)
                        if rows > 64:
                            nc.tensor.matmul(pob[:, c0:c0 + cw],
                                             lhsT=vT_view[64:128, t, :],
                                             rhs=e[64:128, c0:c0 + cw],
                                             start=first_b, stop=last_b)

                # hoist the first pairs of pass 1: their scores+exp run
                # while the v conv and transposes occupy the PE, so ACT
                # is busy through the prep tail
                HOIST = 4
                hoisted = []
                for p in range(HOIST):
                    for t in (2 * p, 2 * p + 1):
                        hoisted.append(
                            (t, scores_exp(0, 1024, t, ITILE, f"h{t}")))

                # --- v conv + transposes ---
                for c0 in range(0, N, 512):
                    conv_chunk(v_sb, w_sb, 512, 576, 2, xin_sb, xW_sb,
                               c0, min(512, N - c0))
                for t in range(NT):
                    tw = ITILE if t < NFULL else TAILW
                    tp_t = mp.tile([128, 64], F32, tag="s", name=f"tp{t}")
                    nc.tensor.matmul(tp_t[0:tw, 0:64],
                                     lhsT=v_sb[0:64, t * ITILE:t * ITILE + tw],
                                     rhs=identb[:, :],
                                     start=True, stop=True)
                    nc.vector.tensor_copy(out=vT_view[0:tw, t, 0:64],
                                          in_=tp_t[0:tw, 0:64])

                # --- main attention loop, software-pipelined so PE never
                #     waits on the current iteration's exp ---
                for (m0, mw) in passes:
                    poa = opp.tile([65, mw], F32, tag="poa", name=f"poa{m0}")
                    pob = opp.tile([65, mw], F32, tag="pob", name=f"pob{m0}")
                    ema = {"a": False, "b": False}  # emitted-first per bank

                    def emit(t, e, rows, last=False):
                        fa, fb = not ema["a"], not ema["b"]
                        out_mms(poa, pob, mw, t, rows, e, fa, fb,
                                last, last)
                        ema["a"] = True
                        if rows > 64:
                            ema["b"] = True

                    pend = []
                    if m0 == 0:
                        for (t, e) in hoisted:
                            emit(t, e, ITILE)
                        p_start = HOIST
                    else:
                        p_start = 0
                    for p in range(p_start, NFULL // 2):
                        tA, tB = 2 * p, 2 * p + 1
                        eA = scores_exp(m0, mw, tA, ITILE, f"A{m0}_{p}")
                        eB = scores_exp(m0, mw, tB, ITILE, f"B{m0}_{p}")
                        pend.append((tA, eA))
                        pend.append((tB, eB))
                        while len(pend) > 4:
                            t0, e0 = pend.pop(0)
                            emit(t0, e0, ITILE)
                    eT = scores_exp(m0, mw, NFULL, TAILW, f"T{m0}")
                    for i, (t0, e0) in enumerate(pend):
                        # bank B's last write is the final full tile
                        fa, fb = not ema["a"], not ema["b"]
                        out_mms(poa, pob, mw, t0, ITILE, e0, fa, fb,
                                False, i == len(pend) - 1)
                        ema["a"] = ema["b"] = True
                    pend = []
                    emit(NFULL, eT, TAILW, last=True)

                    # merge halves + normalize: out = num * (1/den)
                    nb = fin.tile([65, mw], F32, tag="nb", name=f"nb{m0}")
                    nc.vector.tensor_copy(out=nb[:, :], in_=pob[0:65, 0:mw])
                    ns = fin.tile([65, mw], F32, tag="ns", name=f"ns{m0}")
                    nc.vector.tensor_tensor(out=ns[:, :], in0=poa[0:65, 0:mw],
                                            in1=nb[:, :], op=ADD)
                    rc = fin.tile([1, mw], F32, tag="rc", name=f"rc{m0}")
                    nc.vector.reciprocal(rc[:, :], ns[64:65, :])
                    bc = fin.tile([64, mw], F32, tag="bc", name=f"bc{m0}")
                    nc.gpsimd.partition_broadcast(bc[:, :], rc[:, :], channels=64)
                    nc.vector.tensor_tensor(out=out_sb[0:64, m0:m0 + mw],
                                            in0=ns[0:64, :], in1=bc[:, :],
                                            op=MULT)
                nc.sync.dma_start(out_d.ap(), out_sb[:, :])
    nc.compile()
    return nc


def _shifted(xb):
    """xb [C, D, H, W] -> dict of zero-padded unit shifts, flattened [C, N]."""
    z = np.zeros_like(xb)
    sDp = z.copy(); sDp[:, :-1] = xb[:, 1:]
    sDm = z.copy(); sDm[:, 1:] = xb[:, :-1]
    sHp = z.copy(); sHp[:, :, :-1] = xb[:, :, 1:]
    sHm = z.copy(); sHm[:, :, 1:] = xb[:, :, :-1]
    sWp = z.copy(); sWp[..., :-1] = xb[..., 1:]
    sWm = z.copy(); sWm[..., 1:] = xb[..., :-1]
    f = lambda a: a.reshape(C, N)
    return {k: f(v) for k, v in dict(Dp=sDp, Dm=sDm, Hp=sHp, Hm=sHm,
                                     Wp=sWp, Wm=sWm).items()}


def _pack_weights(q_w, k_w, v_w):
    """[128, 640] fp32 lhsT pack. Per conv: 128 cols of center-tap (used slice
    [0:64, base:base+64]) then 128 cols of stacked +/- taps ([0:64
    parts]=plus tap, [64:128]=minus tap). k at 0, q at 256; v center at 512,
    v +/- at 576."""
    kw = k_w[:, :, :, 0, 0]   # [O, I, 3] taps along D
    qw = q_w[:, :, 0, :, 0]   # taps along H
    vw = v_w[:, :, 0, 0, :]   # taps along W
    wp = np.zeros((128, 640), np.float32)

    def put(base, w3):
        wp[0:64, base:base + 64] = np.ascontiguousarray(w3[:, :, 1].T)
        wp[0:64, base + 128:base + 192] = np.ascontiguousarray(w3[:, :, 2].T)
        wp[64:128, base + 128:base + 192] = np.ascontiguousarray(w3[:, :, 0].T)

    put(0, kw)
    put(256, qw)
    wp[0:64, 512:576] = np.ascontiguousarray(vw[:, :, 1].T)
    wp[0:64, 576:640] = np.ascontiguousarray(vw[:, :, 2].T)
    wp[64:128, 576:640] = np.ascontiguousarray(vw[:, :, 0].T)
    return wp


def _pack_bias(q_b, k_b, v_b):
    bp = np.zeros((128, 3), np.float32)
    bp[0:64, 0] = k_b
    bp[0:64, 1] = q_b
    bp[0:64, 2] = v_b
    return bp


def make_in_maps(x, q_w, q_b, k_w, k_b, v_w, v_b):
    x = np.asarray(x, np.float32)
    wp = _pack_weights(np.asarray(q_w, np.float32),
                       np.asarray(k_w, np.float32),
                       np.asarray(v_w, np.float32))
    bp = _pack_bias(np.asarray(q_b, np.float32),
                    np.asarray(k_b, np.float32),
                    np.asarray(v_b, np.float32))
    in_maps = []
    per_batch = []
    for b in range(B):
        xb = x[b]
        x2 = xb.reshape(C, N)
        sh = _shifted(xb)
        per_batch.append((x2, sh))
    for g in range(NCORES):
        b, s = g // CPB, g % CPB
        x2, sh = per_batch[b]
        off = s * MS
        in_maps.append({
            "xin": np.ascontiguousarray(x2, dtype=np.float32),
            "xD": np.ascontiguousarray(np.vstack([sh["Dp"], sh["Dm"]]),
                                       dtype=np.float32),
            "xW": np.ascontiguousarray(np.vstack([sh["Wp"], sh["Wm"]]),
                                       dtype=np.float32),
            "xH": np.ascontiguousarray(
                np.vstack([sh["Hp"][:, off:off + MS],
                           sh["Hm"][:, off:off + MS]]), dtype=np.float32),
            "xq": np.ascontiguousarray(x2[:, off:off + MS], dtype=np.float32),
            "wp": wp,
            "bp": bp,
        })
    return in_maps


def kernel(x, q_w, q_b, k_w, k_b, v_w, v_b, trace=False):
    from concourse.bass_utils import run_bass_kernel_spmd
    if "nc" not in _CACHE:
        _CACHE["nc"] = _build_bass()
    nc = _CACHE["nc"]
    in_maps = make_in_maps(x, q_w, q_b, k_w, k_b, v_w, v_b)
    res = run_bass_kernel_spmd(nc, in_maps, core_ids=list(range(NCORES)),
                               trace=trace)
    _CACHE["last_result"] = res
    out = np.empty((B, C, N), np.float32)
    for g in range(NCORES):
        b, s = g // CPB, g % CPB
        out[b, :, s * MS:(s + 1) * MS] = res.results[g]["out"]
    return out.reshape(B, C, D, H, W)



# revision 23
# speedup vs baseline: 1.2071x; 1.2071x over previous
"""DualAttention Trainium2 kernel.

Problem: x:[2,64,20,20,20]; three separable 1-D convs produce q0 (H-axis),
k0 (D-axis), v (W-axis), each [B,C,N] with N=8000; scores = k0^T q0 [B,N,N];
softmax over the key axis i (axis 1); out = v @ attn, reshaped back.

Sharding: 8 cores = 2 batches x 4 query-column slices of 2000. Each core
computes full k0/v (cheap convs) and its q0 slice, then a flash-style
scores->exp->accumulate loop. No collectives.

Per-core device algorithm (Tile framework):
  - convs as chunked matmuls in float32r (full-rate on the PE at >=256
    moving columns, vs 4 cycles/row for plain fp32): one K=128 matmul for
    the two shifted taps (host pre-stacks shifted x on 128 partitions),
    one K=64 matmul for the center tap. Bias-add + PSUM->SBUF move on the
    ACT engine (Identity w/ per-channel bias; same table set as Exp).
  - scores: bf16 k0/q0 (fp32 PSUM accumulate); i-tile pairs run on
    disjoint PE row groups (k0/q0 duplicated across partition halves) so
    two K=64 matmuls stream concurrently.
  - exp with bias -8 (uniform scale cancels in softmax), bf16 out; 60% of
    i-tiles on ScalarE (exact), 40% on VectorE via a Schraudolph-style
    int16 tensor_scalar whose bits are the bf16 exp approximation
    (~2-3% rel err; softmax num/den errors partly cancel).
  - vT built with regular identity matmuls (cheaper than transpose-mode).
  - second matmul: lhsT = [v^T | 1] bf16; the K=128 contraction is split
    into two K=64 row-group halves that run concurrently, accumulating
    numerator+denominator into two PSUM banks, merged at normalize time.
  - divide by denominator at the end (reciprocal + partition broadcast).
"""

import numpy as np

B, C, D, H, W = 2, 64, 20, 20, 20
N = D * H * W            # 8000
NCORES = 8
CPB = 4                  # cores per batch
MS = N // CPB            # 2000 query columns per core
ITILE = 128
NFULL = N // ITILE       # 62 full i-tiles
TAILW = N - NFULL * ITILE  # 64
NT = NFULL + 1           # 63
M_PASSES = [(0, 1024), (1024, 976)]
EXP_BIAS = -8.0

# Schraudolph-style exp on DVE: int16 bits = SCH_A*s + SCH_B viewed as bf16.
# bf16 bits = 128*log2(v) + 16256; log2(e^(s+EXP_BIAS)) = (s+EXP_BIAS)/ln2.
import math
SCH_A = 128.0 / math.log(2.0)
SCH_C = 7.4
SCH_B = 16256.0 - SCH_C + SCH_A * EXP_BIAS
DVE_TILES = (1, 3)  # t % 5 in this set -> exp on DVE (40%)

# build-time config knobs (A/B testing)
CONFIG = {"bias_act": True, "xw_scalar": True}

_CACHE = {}


def _build_bass(reps=1, bench_loop=0, passes=None):
    if passes is None:
        passes = M_PASSES
    import contextlib
    import concourse.tile as tile
    import concourse.mybir as mybir
    from concourse import bacc
    from concourse.masks import make_identity

    F32 = mybir.dt.float32
    F32R = mybir.dt.float32r
    I16 = mybir.dt.int16
    BF16 = mybir.dt.bfloat16
    EXP = mybir.ActivationFunctionType.Exp
    IDENT = mybir.ActivationFunctionType.Identity
    ADD = mybir.AluOpType.add
    MULT = mybir.AluOpType.mult

    nc = bacc.Bacc("TRN2", target_bir_lowering=False, debug=False,
                   num_devices=NCORES)

    xin_d = nc.dram_tensor("xin", [64, N], F32R, kind="ExternalInput")
    xD_d = nc.dram_tensor("xD", [128, N], F32R, kind="ExternalInput")
    xW_d = nc.dram_tensor("xW", [128, N], F32R, kind="ExternalInput")
    xH_d = nc.dram_tensor("xH", [128, MS], F32R, kind="ExternalInput")
    xq_d = nc.dram_tensor("xq", [64, MS], F32R, kind="ExternalInput")
    wp_d = nc.dram_tensor("wp", [128, 640], F32R, kind="ExternalInput")
    bp_d = nc.dram_tensor("bp", [128, 3], F32, kind="ExternalInput")
    out_d = nc.dram_tensor("out", [64, MS], F32, kind="ExternalOutput")

    PIECE = 2048  # input DMA piece size (512-aligned) for pipelining

    with tile.TileContext(nc) as tc:
        loop_cm = tc.For_i(0, bench_loop, 1) if bench_loop else contextlib.nullcontext()
        with loop_cm:
         for _rep in range(reps):
            with tc.tile_pool(name="persist", bufs=1) as per, \
                 tc.tile_pool(name="mps", bufs=2, space="PSUM") as mp, \
                 tc.tile_pool(name="ops", bufs=1, space="PSUM") as opp, \
                 tc.tile_pool(name="eps", bufs=10) as ep, \
                 tc.tile_pool(name="fin", bufs=1) as fin, \
                 tc.tile_pool(name="prep", bufs=1) as pr:

                w_sb = per.tile([128, 640], F32R, name="w_sb")
                nc.gpsimd.dma_start(w_sb[:, :], wp_d.ap())
                b_sb = per.tile([128, 3], F32, name="b_sb")
                nc.gpsimd.dma_start(b_sb[:, :], bp_d.ap())
                k0_sb = per.tile([128, N], BF16, name="k0_sb")
                q0_sb = per.tile([128, MS], BF16, name="q0_sb")
                v_sb = per.tile([64, N], BF16, name="v_sb")
                vT_sb = per.tile([128, NT * 65], BF16, name="vT_sb")
                out_sb = per.tile([64, MS], F32, name="out_sb")
                identb = per.tile([64, 64], BF16, name="identb")
                expb = per.tile([128, 1], F32, name="expb")

                make_identity(nc, identb[:, :])
                nc.vector.memset(expb[:, :], EXP_BIAS)
                nc.vector.memset(vT_sb[:, :], 1.0)
                vT_view = vT_sb[:, :].rearrange("p (t c) -> p t c", c=65)

                def conv_chunk(dst, wt, w0c, wpmc, bcol, s0_sb, sS_sb, c0, cw):
                    ps = mp.tile([128, 1024], F32, tag="s", name=f"c{w0c}_{c0}")
                    nc.tensor.matmul(ps[0:64, 0:cw],
                                     lhsT=wt[:, wpmc:wpmc + 64],
                                     rhs=sS_sb[:, c0:c0 + cw],
                                     start=True, stop=False)
                    nc.tensor.matmul(ps[0:64, 0:cw],
                                     lhsT=wt[0:64, w0c:w0c + 64],
                                     rhs=s0_sb[0:64, c0:c0 + cw],
                                     start=False, stop=True)
                    # bias-add + PSUM->SBUF move; ACT Identity keeps the
                    # DVE free during prep (exp_and_others set: no switch)
                    if CONFIG["bias_act"]:
                        nc.scalar.activation(dst[0:64, c0:c0 + cw],
                                             ps[0:64, 0:cw], IDENT,
                                             bias=b_sb[0:64, bcol:bcol + 1])
                    else:
                        nc.vector.tensor_scalar(
                            out=dst[0:64, c0:c0 + cw], in0=ps[0:64, 0:cw],
                            scalar1=b_sb[0:64, bcol:bcol + 1], scalar2=None,
                            op0=ADD)

                # --- q conv first (small inputs, unblocks scores) ---
                xH_sb = pr.tile([128, MS], F32R, tag="xh", name="xH_sb")
                nc.sync.dma_start(xH_sb[:, :], xH_d.ap())
                xq_sb = pr.tile([64, MS], F32R, tag="xq", name="xq_sb")
                nc.gpsimd.dma_start(xq_sb[:, :], xq_d.ap())
                for c0 in range(0, MS, 512):
                    conv_chunk(q0_sb, w_sb, 256, 384, 1, xq_sb, xH_sb,
                               c0, min(512, MS - c0))
                nc.scalar.dma_start(q0_sb[64:128, :], q0_sb[0:64, :])

                # --- k conv, pipelined with pieced input DMA; per-chunk dup ---
                xin_sb = pr.tile([64, N], F32R, tag="xin", name="xin_sb")
                xD_sb = pr.tile([128, N], F32R, tag="xd", name="xD_sb")
                for p0 in range(0, N, PIECE):
                    pw = min(PIECE, N - p0)
                    nc.sync.dma_start(xD_sb[:, p0:p0 + pw], xD_d.ap()[:, p0:p0 + pw])
                    nc.gpsimd.dma_start(xin_sb[:, p0:p0 + pw],
                                        xin_d.ap()[:, p0:p0 + pw])
                xW_sb = pr.tile([128, N], F32R, tag="xw", name="xW_sb")
                xw_q = nc.scalar if CONFIG["xw_scalar"] else nc.sync
                for p0 in range(0, N, PIECE):
                    pw = min(PIECE, N - p0)
                    xw_q.dma_start(xW_sb[:, p0:p0 + pw], xW_d.ap()[:, p0:p0 + pw])
                for c0 in range(0, N, 512):
                    cw = min(512, N - c0)
                    conv_chunk(k0_sb, w_sb, 0, 128, 0, xin_sb, xD_sb, c0, cw)
                    nc.gpsimd.dma_start(k0_sb[64:128, c0:c0 + cw],
                                        k0_sb[0:64, c0:c0 + cw])

                def scores_exp(m0, mw, t, rows, name):
                    chunks = [(0, 512), (512, mw - 512)]
                    s = mp.tile([128, mw], F32, tag="s", name=f"s{name}")
                    for (c0, cw) in chunks:
                        nc.tensor.matmul(
                            s[0:rows, c0:c0 + cw],
                            lhsT=k0_sb[t % 2 * 64:t % 2 * 64 + 64,
                                       t * ITILE:t * ITILE + rows],
                            rhs=q0_sb[t % 2 * 64:t % 2 * 64 + 64,
                                      m0 + c0:m0 + c0 + cw],
                            start=True, stop=True)
                    e = ep.tile([128, mw], BF16, tag="e", name=f"e{name}")
                    if t % 5 in DVE_TILES:
                        nc.vector.tensor_scalar(
                            out=e[0:rows, :].bitcast(I16), in0=s[0:rows, :],
                            scalar1=SCH_A, scalar2=SCH_B, op0=MULT, op1=ADD)
                    else:
                        nc.scalar.activation(e[0:rows, :], s[0:rows, :], EXP,
                                             bias=expb[0:rows, :])
                    return e

                def out_mms(poa, pob, mw, t, rows, e, first_a, first_b,
                            last_a, last_b):
                    # K=128 contraction split into two K=64 row-group halves
                    # that run concurrently on the PE; accumulated in two
                    # separate PSUM banks, merged at normalize time.
                    for (c0, cw) in [(0, 512), (512, mw - 512)]:
                        nc.tensor.matmul(poa[:, c0:c0 + cw],
                                         lhsT=vT_view[0:64, t, :],
                                         rhs=e[0:64, c0:c0 + cw],
                                         start=first_a, stop=last_a)
                        if rows > 64:
                            nc.tensor.matmul(pob[:, c0:c0 + cw],
                                             lhsT=vT_view[64:128, t, :],
                                             rhs=e[64:128, c0:c0 + cw],
                                             start=first_b, stop=last_b)

                # hoist the first pairs of pass 1: their scores+exp run
                # while the v conv and transposes occupy the PE, so ACT
                # is busy through the prep tail
                HOIST = 4
                hoisted = []
                for p in range(HOIST):
                    for t in (2 * p, 2 * p + 1):
                        hoisted.append(
                            (t, scores_exp(0, 1024, t, ITILE, f"h{t}")))

                # --- v conv + transposes ---
                for c0 in range(0, N, 512):
                    conv_chunk(v_sb, w_sb, 512, 576, 2, xin_sb, xW_sb,
                               c0, min(512, N - c0))
                for t in range(NT):
                    tw = ITILE if t < NFULL else TAILW
                    tp_t = mp.tile([128, 64], F32, tag="s", name=f"tp{t}")
                    nc.tensor.matmul(tp_t[0:tw, 0:64],
                                     lhsT=v_sb[0:64, t * ITILE:t * ITILE + tw],
                                     rhs=identb[:, :],
                                     start=True, stop=True)
                    nc.vector.tensor_copy(out=vT_view[0:tw, t, 0:64],
                                          in_=tp_t[0:tw, 0:64])

                # --- main attention loop, software-pipelined so PE never
                #     waits on the current iteration's exp ---
                for (m0, mw) in passes:
                    poa = opp.tile([65, mw], F32, tag="poa", name=f"poa{m0}")
                    pob = opp.tile([65, mw], F32, tag="pob", name=f"pob{m0}")
                    ema = {"a": False, "b": False}  # emitted-first per bank

                    def emit(t, e, rows, last=False):
                        fa, fb = not ema["a"], not ema["b"]
                        out_mms(poa, pob, mw, t, rows, e, fa, fb,
                                last, last)
                        ema["a"] = True
                        if rows > 64:
                            ema["b"] = True

                    pend = []
                    if m0 == 0:
                        for (t, e) in hoisted:
                            emit(t, e, ITILE)
                        p_start = HOIST
                    else:
                        p_start = 0
                    for p in range(p_start, NFULL // 2):
                        tA, tB = 2 * p, 2 * p + 1
                        eA = scores_exp(m0, mw, tA, ITILE, f"A{m0}_{p}")
                        eB = scores_exp(m0, mw, tB, ITILE, f"B{m0}_{p}")
                        pend.append((tA, eA))
                        pend.append((tB, eB))
                        while len(pend) > 4:
                            t0, e0 = pend.pop(0)
                            emit(t0, e0, ITILE)
                    eT = scores_exp(m0, mw, NFULL, TAILW, f"T{m0}")
                    for i, (t0, e0) in enumerate(pend):
                        # bank B's last write is the final full tile
                        fa, fb = not ema["a"], not ema["b"]
                        out_mms(poa, pob, mw, t0, ITILE, e0, fa, fb,
                                False, i == len(pend) - 1)
                        ema["a"] = ema["b"] = True
                    pend = []
                    emit(NFULL, eT, TAILW, last=True)

                    # merge halves + normalize: out = num * (1/den)
                    nb = fin.tile([65, mw], F32, tag="nb", name=f"nb{m0}")
                    nc.vector.tensor_copy(out=nb[:, :], in_=pob[0:65, 0:mw])
                    ns = fin.tile([65, mw], F32, tag="ns", name=f"ns{m0}")
                    nc.vector.tensor_tensor(out=ns[:, :], in0=poa[0:65, 0:mw],
                                            in1=nb[:, :], op=ADD)
                    rc = fin.tile([1, mw], F32, tag="rc", name=f"rc{m0}")
                    nc.vector.reciprocal(rc[:, :], ns[64:65, :])
                    bc = fin.tile([64, mw], F32, tag="bc", name=f"bc{m0}")
                    nc.gpsimd.partition_broadcast(bc[:, :], rc[:, :], channels=64)
                    nc.vector.tensor_tensor(out=out_sb[0:64, m0:m0 + mw],
                                            in0=ns[0:64, :], in1=bc[:, :],
                                            op=MULT)
                nc.sync.dma_start(out_d.ap(), out_sb[:, :])
    nc.compile()
    return nc


def _shifted(xb):
    """xb [C, D, H, W] -> dict of zero-padded unit shifts, flattened [C, N]."""
    z = np.zeros_like(xb)
    sDp = z.copy(); sDp[:, :-1] = xb[:, 1:]
    sDm = z.copy(); sDm[:, 1:] = xb[:, :-1]
    sHp = z.copy(); sHp[:, :, :-1] = xb[:, :, 1:]
    sHm = z.copy(); sHm[:, :, 1:] = xb[:, :, :-1]
    sWp = z.copy(); sWp[..., :-1] = xb[..., 1:]
    sWm = z.copy(); sWm[..., 1:] = xb[..., :-1]
    f = lambda a: a.reshape(C, N)
    return {k: f(v) for k, v in dict(Dp=sDp, Dm=sDm, Hp=sHp, Hm=sHm,
                                     Wp=sWp, Wm=sWm).items()}


def _pack_weights(q_w, k_w, v_w):
    """[128, 640] fp32 lhsT pack. Per conv: 128 cols of center-tap (used slice
    [0:64, base:base+64]) then 128 cols of stacked +/- taps ([0:64
    parts]=plus tap, [64:128]=minus tap). k at 0, q at 256; v center at 512,
    v +/- at 576."""
    kw = k_w[:, :, :, 0, 0]   # [O, I, 3] taps along D
    qw = q_w[:, :, 0, :, 0]   # taps along H
    vw = v_w[:, :, 0, 0, :]   # taps along W
    wp = np.zeros((128, 640), np.float32)

    def put(base, w3):
        wp[0:64, base:base + 64] = np.ascontiguousarray(w3[:, :, 1].T)
        wp[0:64, base + 128:base + 192] = np.ascontiguousarray(w3[:, :, 2].T)
        wp[64:128, base + 128:base + 192] = np.ascontiguousarray(w3[:, :, 0].T)

    put(0, kw)
    put(256, qw)
    wp[0:64, 512:576] = np.ascontiguousarray(vw[:, :, 1].T)
    wp[0:64, 576:640] = np.ascontiguousarray(vw[:, :, 2].T)
    wp[64:128, 576:640] = np.ascontiguousarray(vw[:, :, 0].T)
    return wp


def _pack_bias(q_b, k_b, v_b):
    bp = np.zeros((128, 3), np.float32)
    bp[0:64, 0] = k_b
    bp[0:64, 1] = q_b
    bp[0:64, 2] = v_b
    return bp


def make_in_maps(x, q_w, q_b, k_w, k_b, v_w, v_b):
    x = np.asarray(x, np.float32)
    wp = _pack_weights(np.asarray(q_w, np.float32),
                       np.asarray(k_w, np.float32),
                       np.asarray(v_w, np.float32))
    bp = _pack_bias(np.asarray(q_b, np.float32),
                    np.asarray(k_b, np.float32),
                    np.asarray(v_b, np.float32))
    in_maps = []
    per_batch = []
    for b in range(B):
        xb = x[b]
        x2 = xb.reshape(C, N)
        sh = _shifted(xb)
        per_batch.append((x2, sh))
    for g in range(NCORES):
        b, s = g // CPB, g % CPB
        x2, sh = per_batch[b]
        off = s * MS
        in_maps.append({
            "xin": np.ascontiguousarray(x2, dtype=np.float32),
            "xD": np.ascontiguousarray(np.vstack([sh["Dp"], sh["Dm"]]),
                                       dtype=np.float32),
            "xW": np.ascontiguousarray(np.vstack([sh["Wp"], sh["Wm"]]),
                                       dtype=np.float32),
            "xH": np.ascontiguousarray(
                np.vstack([sh["Hp"][:, off:off + MS],
                           sh["Hm"][:, off:off + MS]]), dtype=np.float32),
            "xq": np.ascontiguousarray(x2[:, off:off + MS], dtype=np.float32),
            "wp": wp,
            "bp": bp,
        })
    return in_maps


def kernel(x, q_w, q_b, k_w, k_b, v_w, v_b, trace=False):
    from concourse.bass_utils import run_bass_kernel_spmd
    if "nc" not in _CACHE:
        _CACHE["nc"] = _build_bass()
    nc = _CACHE["nc"]
    in_maps = make_in_maps(x, q_w, q_b, k_w, k_b, v_w, v_b)
    res = run_bass_kernel_spmd(nc, in_maps, core_ids=list(range(NCORES)),
                               trace=trace)
    _CACHE["last_result"] = res
    out = np.empty((B, C, N), np.float32)
    for g in range(NCORES):
        b, s = g // CPB, g % CPB
        out[b, :, s * MS:(s + 1) * MS] = res.results[g]["out"]
    return out.reshape(B, C, D, H, W)

